# revision 19
# baseline (speedup 1.0000x reference)
"""Multi-head causal self-attention (B=64, T=256, C=384, H=6) on 8 NeuronCores.

Data-parallel over batch: each core processes 8 batches (2048 tokens).
Layouts avoid any device-side transposes:
  - xT, Q.T, K.T feature-major [C, tokens]; V token-major [tokens, C]
  - scores computed transposed (S.T[tk, tq]) so exp(S.T) feeds P.T@V directly
  - attention output lands feature-major (catT) for the output projection
Matmul operands bf16; accumulation/softmax fp32; output stored bf16.

vs. 97us baseline:
  - x/weights DMA'd in 512-col chunks, Q/K weights first, so phase 1a can
    start as soon as the first chunks land
  - program is a t-tile pipeline (QK -> V -> proj(t-1) -> attn(2 batches)) so
    big matmuls stay interleaved with the small attention matmuls
  - a PE warm-up burst of dummy matmuls during the input DMA wait pulls the
    HAM clock gate toward 8/8 before the real matmuls arrive
  - softmax denominators are computed ALREADY BROADCAST across the pair's
    128 partitions (ones[128,64] column-packed Z matmuls into the same PSUM
    bank as O.T), so reciprocal+normalize run straight out of PSUM -- no
    DRAM roundtrip for the per-token reciprocals
  - y stored bf16 (halves output DMA traffic; rel-err stays ~3.8e-3)

HW constraint learned the hard way: concurrent row-packed matmuls (both
heads, M=128) must write DIFFERENT PSUM banks -- same-bank same-partition
concurrent PE writes abort the kernel. Column-packed matmuls (O.T/Z) write
disjoint partitions and may share a bank.
"""

import sys

import ml_dtypes
import numpy as np

for _p in ("/opt/trn_rl_repo", "/root/.axon_site/_ro/trn_rl_repo"):
    if _p not in sys.path:
        sys.path.insert(0, _p)

import concourse.bass as bass
import concourse.tile as tile
from concourse import bacc, mybir
from concourse.bass_utils import run_bass_kernel_spmd

B, T, C, H, D = 64, 256, 384, 6, 64
NCORES = 8
BB = B // NCORES  # batches per core = 8
TOK = BB * T      # tokens per core = 2048
SCALE = float(C) ** -0.5
F32 = mybir.dt.float32
BF16 = mybir.dt.bfloat16
NPBF = ml_dtypes.bfloat16

NT4 = TOK // 512  # 4 column-chunks of 512 tokens
NKC = C // 128    # 3 chunks of 128 over feature dim


def build_module():
    nc = bacc.Bacc("TRN2", target_bir_lowering=False, debug=False)

    xT = nc.dram_tensor("xT", [C, TOK], BF16, kind="ExternalInput").ap()
    wall = nc.dram_tensor("wall", [C, 4 * C], BF16, kind="ExternalInput").ap()
    wobc = nc.dram_tensor("wobc", [C, 1], F32, kind="ExternalInput").ap()
    ones = nc.dram_tensor("ones", [128, 64], BF16, kind="ExternalInput").ap()
    yT = nc.dram_tensor("yT", [C, TOK], BF16, kind="ExternalOutput").ap()

    with tile.TileContext(nc) as tc:
        import contextlib

        ctx = contextlib.ExitStack()
        with ctx:
            consts = ctx.enter_context(tc.tile_pool(name="consts", bufs=1))

            def ptile(name, shape, dt=BF16):
                return consts.tile(shape, dt, tag=name, name=name)

            wqk_sb = [ptile(f"wqk{k}", [128, 2 * C]) for k in range(NKC)]
            wvo_sb = [ptile(f"wvo{k}", [128, 2 * C]) for k in range(NKC)]
            wq_sb = [w[:, 0:C] for w in wqk_sb]
            wk_sb = [w[:, C:2 * C] for w in wqk_sb]
            wv_sb = [w[:, 0:C] for w in wvo_sb]
            wo_sb = [w[:, C:2 * C] for w in wvo_sb]
            wob_sb = [ptile(f"wob{k}", [128, 1], F32) for k in range(NKC)]
            ones_sb = ptile("ones", [128, 64])
            xt_sb = [[ptile(f"xt{k}_{t}", [128, 512]) for t in range(NT4)] for k in range(NKC)]
            qt_sb = [[ptile(f"qt{k}_{t}", [128, 512]) for t in range(NT4)] for k in range(NKC)]
            kt_sb = [[ptile(f"kt{k}_{t}", [128, 512]) for t in range(NT4)] for k in range(NKC)]
            cat_sb = [[ptile(f"cat{k}_{t}", [128, 512]) for t in range(NT4)] for k in range(NKC)]
            v_sb = [ptile(f"v{t}", [128, C]) for t in range(2 * BB)]  # 16 token-blocks of 128

            # ---- input DMAs. Each engine queue feeds its own ~80GB/s DMA
            # ring and trigger issue is serial (~640ns each), so balance the
            # transfers k-row-wise across gpsimd/scalar/sync, ordered by
            # when each piece is needed (wqk+x(t0) first, then x(t1), wvo
            # before V(0), x(t2/t3) last). ones leads sync: first Z matmul
            # needs it early and it is tiny.
            ring = [nc.gpsimd, nc.scalar, nc.sync]
            nc.sync.dma_start(out=ones_sb, in_=ones)
            for k in range(NKC):
                ring[k].dma_start(out=wqk_sb[k], in_=wall[128 * k:128 * (k + 1), 0:2 * C])
            for t in range(NT4):
                for k in range(NKC):
                    ring[k].dma_start(
                        out=xt_sb[k][t],
                        in_=xT[128 * k:128 * (k + 1), 512 * t:512 * (t + 1)],
                    )
                if t == 1:
                    for k in range(NKC):
                        ring[k].dma_start(
                            out=wvo_sb[k],
                            in_=wall[128 * k:128 * (k + 1), 2 * C:4 * C])
            for k in range(NKC):
                nc.sync.dma_start(out=wob_sb[k], in_=wobc[128 * k:128 * (k + 1), :])

            # ---- PSUM pools: pa 2x1 + ps 2x2 + poz 2x1 = 8 banks ----
            pa = ctx.enter_context(tc.tile_pool(name="pa", bufs=2, space="PSUM"))
            ps = ctx.enter_context(tc.tile_pool(name="ps", bufs=2, space="PSUM"))
            poz = ctx.enter_context(tc.tile_pool(name="poz", bufs=2, space="PSUM"))

            pt_pool = ctx.enter_context(tc.tile_pool(name="ptp", bufs=BB * H // 2))
            rp_pool = ctx.enter_context(tc.tile_pool(name="rpp", bufs=3))
            y_pool = ctx.enter_context(tc.tile_pool(name="yp", bufs=3))

            def qk(t):
                """Q.T / K.T = W @ x.T for 512-token tile t, drained bf16."""
                for which, w_sb, out_sb in (("q", wq_sb, qt_sb), ("k", wk_sb, kt_sb)):
                    for co in range(NKC):
                        pqk = pa.tile([128, 512], F32, tag="pa", name=f"p{which}{co}_{t}")
                        for kc in range(NKC):
                            nc.tensor.matmul(
                                pqk,
                                w_sb[kc][:, 128 * co:128 * (co + 1)],
                                xt_sb[kc][t],
                                start=(kc == 0),
                                stop=(kc == NKC - 1),
                            )
                        nc.vector.tensor_copy(out_sb[co][t], pqk)

            def vproj(t):
                """V token-major [tok, C] for the 4 token-blocks of tile t."""
                for j in range(4):
                    tb = 4 * t + j
                    pv = pa.tile([128, C], F32, tag="pa", name=f"pv{tb}")
                    for kc in range(NKC):
                        nc.tensor.matmul(
                            pv,
                            xt_sb[kc][t][:, 128 * j:128 * (j + 1)],
                            wv_sb[kc],
                            start=(kc == 0),
                            stop=(kc == NKC - 1),
                        )
                    nc.scalar.copy(v_sb[tb], pv)

            def attn(b):
                """Attention for batch b, head pairs. Score layout per pair:
                cols 0:512   = 4 causal-diagonal 128-blocks [h0b0|h0b1|h1b0|h1b1]
                cols 512:768 = full (strictly lower) blocks [h0|h1]"""
                t4b, qc = b // 2, (b % 2) * 256
                for hp in range(H // 2):
                    kt = kt_sb[hp][t4b]
                    qt = qt_sb[hp][t4b]
                    # scores: head hh lives entirely in PSUM bank hh of the
                    # tile (concurrent row-packed matmuls must not write the
                    # same bank at the same partitions). Per head:
                    # cols +0:256 = tk-blk0 x tq 0:256, +256:384 = tk-blk1 x
                    # tq 128:256.
                    p_s = ps.tile([128, 1024], F32, tag="ps", name=f"s{b}_{hp}")
                    for hh in range(2):
                        r0, sb = 64 * hh, 512 * hh
                        nc.tensor.matmul(
                            p_s[:, sb:sb + 256],
                            kt[r0:r0 + 64, qc:qc + 128],
                            qt[r0:r0 + 64, qc:qc + 256],
                            start=True, stop=True,
                        )
                        nc.tensor.matmul(
                            p_s[:, sb + 256:sb + 384],
                            kt[r0:r0 + 64, qc + 128:qc + 256],
                            qt[r0:r0 + 64, qc + 128:qc + 256],
                            start=True, stop=True,
                        )
                    # P.T = exp(S.T / sqrt(C)) for both heads in one op
                    pt = pt_pool.tile([128, 768], BF16, tag="pt", name=f"pt{b}_{hp}")
                    nc.scalar.activation(
                        pt.rearrange("p (a q) -> p a q", q=384),
                        p_s.rearrange("p (a q) -> p a q", q=512)[:, :, 0:384],
                        mybir.ActivationFunctionType.Exp, scale=SCALE,
                    )
                    # causal mask on the diagonal blocks (2 free dims max)
                    for hh in range(2):
                        sel = pt[:, 384 * hh:384 * (hh + 1)] \
                            .rearrange("p (c i) -> p c i", i=128)[:, 0::2, :]
                        nc.gpsimd.affine_select(
                            out=sel, in_=sel,
                            pattern=[[0, 2], [1, 128]],
                            compare_op=mybir.AluOpType.is_ge,
                            fill=0.0, base=0, channel_multiplier=-1,
                        )
                    # O.T (cols 0:256) and broadcast Z (cols 256:512) in one
                    # bank; col-packed writes are partition-disjoint -> safe
                    po = poz.tile([128, 512], F32, tag="poz", name=f"poz{b}_{hp}")
                    for hh in range(2):
                        h = 2 * hp + hh
                        r0, pb = 64 * hh, 384 * hh
                        nc.tensor.matmul(
                            po[r0:r0 + 64, 0:256],
                            v_sb[2 * b][:, 64 * h:64 * (h + 1)],
                            pt[:, pb:pb + 256],
                            start=True, stop=False,
                            tile_position=(0, r0), skip_group_check=True,
                        )
                        nc.tensor.matmul(
                            po[r0:r0 + 64, 128:256],
                            v_sb[2 * b + 1][:, 64 * h:64 * (h + 1)],
                            pt[:, pb + 256:pb + 384],
                            start=False, stop=True,
                            tile_position=(0, r0), skip_group_check=True,
                        )
                    for hh in range(2):
                        r0, pb = 64 * hh, 384 * hh
                        nc.tensor.matmul(
                            po[r0:r0 + 64, 256:512],
                            ones_sb, pt[:, pb:pb + 256],
                            start=True, stop=False,
                            tile_position=(0, r0), skip_group_check=True,
                        )
                        nc.tensor.matmul(
                            po[r0:r0 + 64, 384:512],
                            ones_sb, pt[:, pb + 256:pb + 384],
                            start=False, stop=True,
                            tile_position=(0, r0), skip_group_check=True,
                        )
                    # normalize: cat = O.T * (1/Z). Safe to read only the Z
                    # half: the PE completes matmuls in program order, so by
                    # the time the Z writes are done the O writes are too.
                    rp = rp_pool.tile([128, 256], F32, tag="rp", name=f"rp{b}_{hp}")
                    nc.vector.reciprocal_approx_fast(rp, po[:, 256:512])
                    nc.vector.tensor_mul(
                        cat_sb[hp][t4b][:, qc:qc + 256], po[:, 0:256], rp,
                    )

            def proj(t):
                """y.T = Wo @ catT + bo for tile t, stored bf16."""
                for co in range(NKC):
                    pyk = pa.tile([128, 512], F32, tag="pa", name=f"py{co}_{t}")
                    for kc in range(NKC):
                        nc.tensor.matmul(
                            pyk,
                            wo_sb[kc][:, 128 * co:128 * (co + 1)],
                            cat_sb[kc][t],
                            start=(kc == 0),
                            stop=(kc == NKC - 1),
                        )
                    yt = y_pool.tile([128, 512], BF16, tag="yt", name=f"yt{co}_{t}")
                    nc.scalar.add(yt, pyk, wob_sb[co][:, 0:1])
                    nc.sync.dma_start(
                        out=yT[128 * co:128 * (co + 1), 512 * t:512 * (t + 1)],
                        in_=yt,
                    )

            # ---- PE warm-up: dummy matmuls on a memset tile so the HAM
            # clock gate reaches 8/8 before the real matmuls arrive ----
            wtile = consts.tile([128, 128], BF16, tag="warm", name="warm")
            nc.vector.memset(wtile, 0.0)
            wz = pa.tile([128, 128], F32, tag="pa", name="warmz")
            for i in range(100):
                nc.tensor.matmul(wz[:, 0:64], wtile, wtile[:, 0:64], start=True, stop=True)

            for t in range(NT4):
                qk(t)
                vproj(t)
                if t > 0:
                    proj(t - 1)
                attn(2 * t)
                attn(2 * t + 1)
            proj(NT4 - 1)

    nc.compile()
    return nc


def make_in_maps(x, Wk, Wq, Wv, Wo, bo):
    x = np.asarray(x, np.float32)
    wall = np.concatenate(
        [np.asarray(w, np.float32).T for w in (Wq, Wk, Wv, Wo)], axis=1
    ).astype(NPBF)
    wobc = np.ascontiguousarray(np.asarray(bo, np.float32).reshape(C, 1))
    ones = np.ones((128, 64), NPBF)
    in_maps = []
    for i in range(NCORES):
        xi = x[BB * i:BB * (i + 1)].reshape(TOK, C)
        in_maps.append({
            "xT": np.ascontiguousarray(xi.T).astype(NPBF),
            "wall": wall, "wobc": wobc, "ones": ones,
        })
    return in_maps


_NC_CACHE = None


def kernel(x, Wk, Wq, Wv, Wo, bo):
    global _NC_CACHE
    if _NC_CACHE is None:
        _NC_CACHE = build_module()
    nc = _NC_CACHE
    in_maps = make_in_maps(x, Wk, Wq, Wv, Wo, bo)
    res = run_bass_kernel_spmd(nc, in_maps, core_ids=list(range(NCORES)))
    outs = []
    for i in range(NCORES):
        yt = np.asarray(res.results[i]["yT"]).astype(np.float32)
        outs.append(yt.T.reshape(BB, T, C))
    return np.concatenate(outs, axis=0).astype(np.float32)


# revision 21
# speedup vs baseline: 1.1788x; 1.1788x over previous
"""Multi-head causal self-attention (B=64, T=256, C=384, H=6) on 8 NeuronCores.

Data-parallel over batch: each core processes 8 batches (2048 tokens).
Layouts avoid any device-side transposes:
  - xT, Q.T, K.T feature-major [C, tokens]; V token-major [tokens, C]
  - scores computed transposed (S.T[tk, tq]) so exp(S.T) feeds P.T@V directly
  - attention output lands feature-major (catT) for the output projection
Matmul operands bf16; accumulation/softmax fp32; output stored bf16.

vs. 97us baseline:
  - x/weights DMA'd in 512-col chunks, Q/K weights first, so phase 1a can
    start as soon as the first chunks land
  - program is a t-tile pipeline (QK -> V -> proj(t-1) -> attn(2 batches)) so
    big matmuls stay interleaved with the small attention matmuls
  - a PE warm-up burst of dummy matmuls during the input DMA wait pulls the
    HAM clock gate toward 8/8 before the real matmuls arrive
  - softmax denominators are computed ALREADY BROADCAST across the pair's
    128 partitions (ones[128,64] column-packed Z matmuls into the same PSUM
    bank as O.T), so reciprocal+normalize run straight out of PSUM -- no
    DRAM roundtrip for the per-token reciprocals
  - y stored bf16 (halves output DMA traffic; rel-err stays ~3.8e-3)

HW constraint learned the hard way: concurrent row-packed matmuls (both
heads, M=128) must write DIFFERENT PSUM banks -- same-bank same-partition
concurrent PE writes abort the kernel. Column-packed matmuls (O.T/Z) write
disjoint partitions and may share a bank.
"""

import sys

import ml_dtypes
import numpy as np

for _p in ("/opt/trn_rl_repo", "/root/.axon_site/_ro/trn_rl_repo"):
    if _p not in sys.path:
        sys.path.insert(0, _p)

import concourse.bass as bass
import concourse.tile as tile
from concourse import bacc, mybir
from concourse.bass_utils import run_bass_kernel_spmd

B, T, C, H, D = 64, 256, 384, 6, 64
NCORES = 8
BB = B // NCORES  # batches per core = 8
TOK = BB * T      # tokens per core = 2048
SCALE = float(C) ** -0.5
F32 = mybir.dt.float32
BF16 = mybir.dt.bfloat16
NPBF = ml_dtypes.bfloat16

NT4 = TOK // 512  # 4 column-chunks of 512 tokens
NKC = C // 128    # 3 chunks of 128 over feature dim


def build_module():
    nc = bacc.Bacc("TRN2", target_bir_lowering=False, debug=False)

    xT = nc.dram_tensor("xT", [C, TOK], BF16, kind="ExternalInput").ap()
    wall = nc.dram_tensor("wall", [C, 4 * C], BF16, kind="ExternalInput").ap()
    wobc = nc.dram_tensor("wobc", [C, 1], F32, kind="ExternalInput").ap()
    ones = nc.dram_tensor("ones", [128, 64], BF16, kind="ExternalInput").ap()
    yT = nc.dram_tensor("yT", [C, TOK], BF16, kind="ExternalOutput").ap()

    with tile.TileContext(nc) as tc:
        import contextlib

        ctx = contextlib.ExitStack()
        with ctx:
            consts = ctx.enter_context(tc.tile_pool(name="consts", bufs=1))

            def ptile(name, shape, dt=BF16):
                return consts.tile(shape, dt, tag=name, name=name)

            wqk_sb = [ptile(f"wqk{k}", [128, 2 * C]) for k in range(NKC)]
            wvo_sb = [ptile(f"wvo{k}", [128, 2 * C]) for k in range(NKC)]
            wq_sb = [w[:, 0:C] for w in wqk_sb]
            wk_sb = [w[:, C:2 * C] for w in wqk_sb]
            wv_sb = [w[:, 0:C] for w in wvo_sb]
            wo_sb = [w[:, C:2 * C] for w in wvo_sb]
            wob_sb = [ptile(f"wob{k}", [128, 1], F32) for k in range(NKC)]
            ones_sb = ptile("ones", [128, 64])
            xt_sb = [[ptile(f"xt{k}_{t}", [128, 512]) for t in range(NT4)] for k in range(NKC)]
            qt_sb = [[ptile(f"qt{k}_{t}", [128, 512]) for t in range(NT4)] for k in range(NKC)]
            kt_sb = [[ptile(f"kt{k}_{t}", [128, 512]) for t in range(NT4)] for k in range(NKC)]
            cat_sb = [[ptile(f"cat{k}_{t}", [128, 512]) for t in range(NT4)] for k in range(NKC)]
            v_sb = [ptile(f"v{t}", [128, C]) for t in range(2 * BB)]  # 16 token-blocks of 128

            # ---- input DMAs. Each engine queue feeds its own ~80GB/s DMA
            # ring and trigger issue is serial (~640ns each), so balance the
            # transfers k-row-wise across gpsimd/scalar/sync, ordered by
            # when each piece is needed (wqk+x(t0) first, then x(t1), wvo
            # before V(0), x(t2/t3) last). ones leads sync: first Z matmul
            # needs it early and it is tiny.
            ring = [nc.gpsimd, nc.scalar, nc.sync]
            nc.sync.dma_start(out=ones_sb, in_=ones)
            for k in range(NKC):
                ring[k].dma_start(out=wqk_sb[k], in_=wall[128 * k:128 * (k + 1), 0:2 * C])
            for t in range(NT4):
                for k in range(NKC):
                    ring[k].dma_start(
                        out=xt_sb[k][t],
                        in_=xT[128 * k:128 * (k + 1), 512 * t:512 * (t + 1)],
                    )
                if t == 1:
                    for k in range(NKC):
                        ring[k].dma_start(
                            out=wvo_sb[k],
                            in_=wall[128 * k:128 * (k + 1), 2 * C:4 * C])
            for k in range(NKC):
                nc.sync.dma_start(out=wob_sb[k], in_=wobc[128 * k:128 * (k + 1), :])

            # ---- PSUM pools: pa 2x1 + ps 2x2 + poz 2x1 = 8 banks ----
            pa = ctx.enter_context(tc.tile_pool(name="pa", bufs=2, space="PSUM"))
            ps = ctx.enter_context(tc.tile_pool(name="ps", bufs=2, space="PSUM"))
            poz = ctx.enter_context(tc.tile_pool(name="poz", bufs=2, space="PSUM"))

            pt_pool = ctx.enter_context(tc.tile_pool(name="ptp", bufs=BB * H // 2))
            rp_pool = ctx.enter_context(tc.tile_pool(name="rpp", bufs=3))
            y_pool = ctx.enter_context(tc.tile_pool(name="yp", bufs=3))

            def qk(t):
                """Q.T / K.T = W @ x.T for 512-token tile t, drained bf16."""
                for which, w_sb, out_sb in (("q", wq_sb, qt_sb), ("k", wk_sb, kt_sb)):
                    for co in range(NKC):
                        pqk = pa.tile([128, 512], F32, tag="pa", name=f"p{which}{co}_{t}")
                        for kc in range(NKC):
                            nc.tensor.matmul(
                                pqk,
                                w_sb[kc][:, 128 * co:128 * (co + 1)],
                                xt_sb[kc][t],
                                start=(kc == 0),
                                stop=(kc == NKC - 1),
                            )
                        nc.vector.tensor_copy(out_sb[co][t], pqk)

            def vproj(t):
                """V token-major [tok, C] for the 4 token-blocks of tile t."""
                for j in range(4):
                    tb = 4 * t + j
                    pv = pa.tile([128, C], F32, tag="pa", name=f"pv{tb}")
                    for kc in range(NKC):
                        nc.tensor.matmul(
                            pv,
                            xt_sb[kc][t][:, 128 * j:128 * (j + 1)],
                            wv_sb[kc],
                            start=(kc == 0),
                            stop=(kc == NKC - 1),
                        )
                    nc.scalar.copy(v_sb[tb], pv)

            def attn(b):
                """Attention for batch b, head pairs. Score layout per pair:
                cols 0:512   = 4 causal-diagonal 128-blocks [h0b0|h0b1|h1b0|h1b1]
                cols 512:768 = full (strictly lower) blocks [h0|h1]"""
                t4b, qc = b // 2, (b % 2) * 256
                for hp in range(H // 2):
                    kt = kt_sb[hp][t4b]
                    qt = qt_sb[hp][t4b]
                    # scores: head hh lives entirely in PSUM bank hh of the
                    # tile (concurrent row-packed matmuls must not write the
                    # same bank at the same partitions). Per head:
                    # cols +0:256 = tk-blk0 x tq 0:256, +256:384 = tk-blk1 x
                    # tq 128:256.
                    p_s = ps.tile([128, 1024], F32, tag="ps", name=f"s{b}_{hp}")
                    for hh in range(2):
                        r0, sb = 64 * hh, 512 * hh
                        nc.tensor.matmul(
                            p_s[:, sb:sb + 256],
                            kt[r0:r0 + 64, qc:qc + 128],
                            qt[r0:r0 + 64, qc:qc + 256],
                            start=True, stop=True,
                        )
                        nc.tensor.matmul(
                            p_s[:, sb + 256:sb + 384],
                            kt[r0:r0 + 64, qc + 128:qc + 256],
                            qt[r0:r0 + 64, qc + 128:qc + 256],
                            start=True, stop=True,
                        )
                    # P.T = exp(S.T / sqrt(C)) for both heads in one op
                    pt = pt_pool.tile([128, 768], BF16, tag="pt", name=f"pt{b}_{hp}")
                    nc.scalar.activation(
                        pt.rearrange("p (a q) -> p a q", q=384),
                        p_s.rearrange("p (a q) -> p a q", q=512)[:, :, 0:384],
                        mybir.ActivationFunctionType.Exp, scale=SCALE,
                    )
                    # causal mask on the diagonal blocks (2 free dims max)
                    for hh in range(2):
                        sel = pt[:, 384 * hh:384 * (hh + 1)] \
                            .rearrange("p (c i) -> p c i", i=128)[:, 0::2, :]
                        nc.gpsimd.affine_select(
                            out=sel, in_=sel,
                            pattern=[[0, 2], [1, 128]],
                            compare_op=mybir.AluOpType.is_ge,
                            fill=0.0, base=0, channel_multiplier=-1,
                        )
                    # O.T (cols 0:256) and broadcast Z (cols 256:512) in one
                    # bank; col-packed writes are partition-disjoint -> safe
                    po = poz.tile([128, 512], F32, tag="poz", name=f"poz{b}_{hp}")
                    for hh in range(2):
                        h = 2 * hp + hh
                        r0, pb = 64 * hh, 384 * hh
                        nc.tensor.matmul(
                            po[r0:r0 + 64, 0:256],
                            v_sb[2 * b][:, 64 * h:64 * (h + 1)],
                            pt[:, pb:pb + 256],
                            start=True, stop=False,
                            tile_position=(0, r0), skip_group_check=True,
                        )
                        nc.tensor.matmul(
                            po[r0:r0 + 64, 128:256],
                            v_sb[2 * b + 1][:, 64 * h:64 * (h + 1)],
                            pt[:, pb + 256:pb + 384],
                            start=False, stop=True,
                            tile_position=(0, r0), skip_group_check=True,
                        )
                    for hh in range(2):
                        r0, pb = 64 * hh, 384 * hh
                        nc.tensor.matmul(
                            po[r0:r0 + 64, 256:512],
                            ones_sb, pt[:, pb:pb + 256],
                            start=True, stop=False,
                            tile_position=(0, r0), skip_group_check=True,
                        )
                        nc.tensor.matmul(
                            po[r0:r0 + 64, 384:512],
                            ones_sb, pt[:, pb + 256:pb + 384],
                            start=False, stop=True,
                            tile_position=(0, r0), skip_group_check=True,
                        )
                    # normalize: cat = O.T * (1/Z). Safe to read only the Z
                    # half: the PE completes matmuls in program order, so by
                    # the time the Z writes are done the O writes are too.
                    rp = rp_pool.tile([128, 256], F32, tag="rp", name=f"rp{b}_{hp}")
                    nc.vector.reciprocal_approx_fast(rp, po[:, 256:512])
                    nc.vector.tensor_mul(
                        cat_sb[hp][t4b][:, qc:qc + 256], po[:, 0:256], rp,
                    )

            def proj(t):
                """y.T = Wo @ catT + bo for tile t, stored bf16."""
                for co in range(NKC):
                    pyk = pa.tile([128, 512], F32, tag="pa", name=f"py{co}_{t}")
                    for kc in range(NKC):
                        nc.tensor.matmul(
                            pyk,
                            wo_sb[kc][:, 128 * co:128 * (co + 1)],
                            cat_sb[kc][t],
                            start=(kc == 0),
                            stop=(kc == NKC - 1),
                        )
                    yt = y_pool.tile([128, 512], BF16, tag="yt", name=f"yt{co}_{t}")
                    nc.scalar.add(yt, pyk, wob_sb[co][:, 0:1])
                    nc.sync.dma_start(
                        out=yT[128 * co:128 * (co + 1), 512 * t:512 * (t + 1)],
                        in_=yt,
                    )

            # ---- PE warm-up: dummy matmuls on a memset tile so the HAM
            # clock gate reaches 8/8 before the real matmuls arrive ----
            wtile = consts.tile([128, 128], BF16, tag="warm", name="warm")
            nc.vector.memset(wtile, 0.0)
            wz = pa.tile([128, 128], F32, tag="pa", name="warmz")
            for i in range(100):
                nc.tensor.matmul(wz[:, 0:64], wtile, wtile[:, 0:64], start=True, stop=True)

            def proj_half(t, half):
                """Half-tile projection (one batch's 256 tokens) for the last
                tile: halves the dependency chain after the final attention."""
                qc = 256 * half
                for co in range(NKC):
                    pyk = pa.tile([128, 256], F32, tag="pa", name=f"pyh{co}_{half}")
                    for kc in range(NKC):
                        nc.tensor.matmul(
                            pyk,
                            wo_sb[kc][:, 128 * co:128 * (co + 1)],
                            cat_sb[kc][t][:, qc:qc + 256],
                            start=(kc == 0),
                            stop=(kc == NKC - 1),
                        )
                    yt = y_pool.tile([128, 256], BF16, tag="yt", name=f"yth{co}_{half}")
                    nc.scalar.add(yt, pyk, wob_sb[co][:, 0:1])
                    nc.sync.dma_start(
                        out=yT[128 * co:128 * (co + 1),
                               512 * t + qc:512 * t + qc + 256],
                        in_=yt,
                    )

            for t in range(NT4):
                qk(t)
                vproj(t)
                attn(2 * t)
                # proj(t-1) after attn(2t): its matmuls wait on the previous
                # iteration's cat normalize, and emitting them first would
                # block ready attention matmuls in the PE queue
                if t > 0:
                    proj(t - 1)
                if t == NT4 - 1:
                    proj_half(t, 0)
                attn(2 * t + 1)
            proj_half(NT4 - 1, 1)

    nc.compile()
    return nc


def make_in_maps(x, Wk, Wq, Wv, Wo, bo):
    x = np.asarray(x, np.float32)
    wall = np.concatenate(
        [np.asarray(w, np.float32).T for w in (Wq, Wk, Wv, Wo)], axis=1
    ).astype(NPBF)
    wobc = np.ascontiguousarray(np.asarray(bo, np.float32).reshape(C, 1))
    ones = np.ones((128, 64), NPBF)
    in_maps = []
    for i in range(NCORES):
        xi = x[BB * i:BB * (i + 1)].reshape(TOK, C)
        in_maps.append({
            "xT": np.ascontiguousarray(xi.T).astype(NPBF),
            "wall": wall, "wobc": wobc, "ones": ones,
        })
    return in_maps


_NC_CACHE = None


def kernel(x, Wk, Wq, Wv, Wo, bo):
    global _NC_CACHE
    if _NC_CACHE is None:
        _NC_CACHE = build_module()
    nc = _NC_CACHE
    in_maps = make_in_maps(x, Wk, Wq, Wv, Wo, bo)
    res = run_bass_kernel_spmd(nc, in_maps, core_ids=list(range(NCORES)))
    outs = []
    for i in range(NCORES):
        yt = np.asarray(res.results[i]["yT"]).astype(np.float32)
        outs.append(yt.T.reshape(BB, T, C))
    return np.concatenate(outs, axis=0).astype(np.float32)
